# revision 16
# baseline (speedup 1.0000x reference)
"""Trainium2 Bass kernel for nn_EnsembleE2EModule (moe_routing).

Strategy: fully data-parallel over batch B=1024 across 8 cores (128 samples
per core). Each core computes, for its batch shard:
  - cos_sim / cos_dist against all 128 keys (fp32 matmul)
  - the 400-iteration Sinkhorn soft-top-k, reduced EXACTLY to a scalar
    per-sample recursion:  P = sum_i sigmoid(Delta + z_i)  (one ACT
    sigmoid+accumulate op per iteration), then
    Delta' = Delta + log(k/(N-k)) + ln(N/P - 1)  with the ln computed on the
    vector engine in software (exponent/mantissa split + deg-6 polynomial).
    knn_sim = sigmoid(Delta_400 + z) elementwise (the final sigmoid output).
  - all 128 experts' logits x @ W_e^T via fp32r matmuls (weights streamed
    from HBM pre-transposed on host), drained PSUM->SBUF, tanh'd, then gated
    and summed over experts with diagonal-matmul accumulation in PSUM.
  - vanilla (log_softmax) and tanh classifiers.
Host side only shards/transposes inputs and concatenates per-core outputs.
"""

import os
import sys

import numpy as np

sys.path.insert(0, "/opt/trn_rl_repo")

import concourse.bass as bass  # noqa: E402
import concourse.bacc as bacc  # noqa: E402
import concourse.mybir as mybir  # noqa: E402
import concourse.tile as tile  # noqa: E402

A = mybir.ActivationFunctionType
ALU = mybir.AluOpType
F32 = mybir.dt.float32
F32R = mybir.dt.float32r
I32 = mybir.dt.int32

B, D, E, C = 1024, 512, 128, 256
NCORES = 8
BS = B // NCORES            # 128 samples per core
N = E                       # sinkhorn column count
K = 16
EPS = 5e-4
ITERS = 400
SQ = float(np.float32(np.sqrt(1.0 / EPS)))        # 44.72136
LOGR = float(np.float32(np.log(K / (N - K))))     # ln(16/112)
LN2 = float(np.float32(np.log(2.0)))

# deg-6 polynomial for ln(1+u) on [0,1), highest degree first (~3.5e-6 abs err)
_u = np.linspace(0, 1, 20001)
_P6 = np.polynomial.chebyshev.Chebyshev.fit(_u, np.log1p(_u), 6).convert(
    kind=np.polynomial.Polynomial
)
LNC = [float(np.float32(v)) for v in _P6.coef[::-1]]
# raw (biased) exponent is used in the final fma; fold -127*ln2 into the
# polynomial's constant term so the exponent op needs no arithmetic fixup
LNC[-1] = float(np.float32(LNC[-1] - 127.0 * np.log(2.0)))

LAST_RESULTS = None


def build_nc(with_expert_bias=False):
    nc = bacc.Bacc(
        "TRN2", target_bir_lowering=False, debug=False, enable_asserts=False,
        num_devices=NCORES,
    )

    # ---- I/O ----
    i_xT = nc.dram_tensor("xT", [D, BS], F32, kind="ExternalInput")
    i_x = nc.dram_tensor("x", [BS, D], F32, kind="ExternalInput")
    i_kT = nc.dram_tensor("keysT", [D, E], F32, kind="ExternalInput")
    i_ewT = nc.dram_tensor("ewT", [E, D, C], F32, kind="ExternalInput")
    i_eb = nc.dram_tensor("eb", [E, C], F32, kind="ExternalInput")
    i_vwT = nc.dram_tensor("vwT", [D, C], F32, kind="ExternalInput")
    i_vb = nc.dram_tensor("vb", [1, C], F32, kind="ExternalInput")
    i_twT = nc.dram_tensor("twT", [D, C], F32, kind="ExternalInput")
    i_tb = nc.dram_tensor("tb", [1, C], F32, kind="ExternalInput")
    i_id = nc.dram_tensor("iden10", [BS, BS], F32, kind="ExternalInput")

    o_ens = nc.dram_tensor("o_ens", [BS, C], F32, kind="ExternalOutput")
    o_tanh = nc.dram_tensor("o_tanh", [BS, C], F32, kind="ExternalOutput")
    o_van = nc.dram_tensor("o_van", [BS, C], F32, kind="ExternalOutput")
    o_cd = nc.dram_tensor("o_cd", [BS, N], F32, kind="ExternalOutput")
    o_knn = nc.dram_tensor("o_knn", [BS, N], F32, kind="ExternalOutput")

    with tile.TileContext(nc) as tc:
        with (
            tc.tile_pool(name="consts", bufs=1) as cst,
            tc.tile_pool(name="ew", bufs=4) as ewp,
            tc.tile_pool(name="store", bufs=1) as stp,
            tc.tile_pool(name="work", bufs=2) as wk,
            tc.tile_pool(name="small", bufs=2) as sm,
            tc.tile_pool(name="diagp", bufs=3) as dgp,
            tc.tile_pool(name="pc", bufs=1, space="PSUM") as ppc,
            tc.tile_pool(name="pcls", bufs=2, space="PSUM") as pcls,
            tc.tile_pool(name="pe", bufs=4, space="PSUM") as pep,
            tc.tile_pool(name="pens", bufs=1, space="PSUM") as pnp,
        ):
            # ---------------- constant loads ----------------
            xt = cst.tile([128, 4, BS], F32)       # xT as 4 d-chunks
            nc.sync.dma_start(xt[:], i_xT.ap().rearrange("(a p) c -> p a c", p=128))
            xn = cst.tile([BS, D], F32)
            nc.sync.dma_start(xn[:], i_x[:, :])
            kt = cst.tile([128, 4, E], F32)
            nc.sync.dma_start(kt[:], i_kT.ap().rearrange("(a p) c -> p a c", p=128))
            vw = cst.tile([128, 4, C], F32)
            nc.sync.dma_start(vw[:], i_vwT.ap().rearrange("(a p) c -> p a c", p=128))
            tw = cst.tile([128, 4, C], F32)
            nc.sync.dma_start(tw[:], i_twT.ap().rearrange("(a p) c -> p a c", p=128))
            vb1 = cst.tile([1, C], F32)
            nc.sync.dma_start(vb1[:], i_vb[:, :])
            tb1 = cst.tile([1, C], F32)
            nc.sync.dma_start(tb1[:], i_tb[:, :])
            iden = cst.tile([BS, BS], F32)
            nc.sync.dma_start(iden[:], i_id[:, :])
            ones1 = cst.tile([1, BS], F32)
            nc.vector.memset(ones1[:], 1.0)

            # ---------------- row norms of x ----------------
            sq_scr = wk.tile([BS, D], F32, tag="sq")
            sqn = cst.tile([BS, 1], F32)
            nc.scalar.activation(sq_scr[:], xn[:], A.Square, accum_out=sqn[:])
            ln_s = cst.tile([BS, 1], F32)
            nc.scalar.activation(ln_s[:], sqn[:], A.Ln)
            rnorm = cst.tile([BS, 1], F32)
            nc.scalar.activation(rnorm[:], ln_s[:], A.Exp, scale=-0.5)

            # ---------------- cos_sim / cos_dist / z ----------------
            pcos = ppc.tile([BS, E], F32)
            for dc in range(4):
                nc.tensor.matmul(pcos[:], xt[:, dc, :], kt[:, dc, :],
                                 start=(dc == 0), stop=(dc == 3))
            cos = cst.tile([BS, E], F32)
            nc.scalar.activation(cos[:], pcos[:], A.Copy, scale=rnorm[:])
            cd = cst.tile([BS, E], F32)
            nc.vector.tensor_scalar(cd[:], cos[:], -1.0, 1.0, ALU.mult, ALU.add)
            nc.sync.dma_start(o_cd[:, :], cd[:])
            rowmax = sm.tile([BS, 1], F32, tag="s1")
            nc.vector.tensor_reduce(rowmax[:], cd[:], mybir.AxisListType.X, ALU.max)
            invmax = sm.tile([BS, 1], F32, tag="s2")
            nc.vector.reciprocal(invmax[:], rowmax[:])
            dh = cst.tile([BS, E], F32)
            nc.vector.tensor_scalar(dh[:], cd[:], invmax[:], SQ, ALU.mult, ALU.mult)
            c0 = cst.tile([BS, E], F32)
            nc.scalar.activation(c0[:], dh[:], A.Square)
            dh2 = cst.tile([BS, E], F32)
            nc.vector.tensor_scalar(dh2[:], dh[:], SQ, None, ALU.subtract)
            c1 = cst.tile([BS, E], F32)
            nc.scalar.activation(c1[:], dh2[:], A.Square)
            zt = cst.tile([BS, E], F32)
            nc.vector.tensor_tensor(zt[:], c1[:], c0[:], ALU.subtract)

            # ---------------- sinkhorn init: Delta_1 ----------------
            m0 = sm.tile([BS, 1], F32, tag="s1")
            nc.vector.tensor_reduce(m0[:], c0[:], mybir.AxisListType.X, ALU.min)
            m1 = sm.tile([BS, 1], F32, tag="s2")
            nc.vector.tensor_reduce(m1[:], c1[:], mybir.AxisListType.X, ALU.min)
            pk = sm.tile([BS, 2], F32, tag="s3")
            e_scr = wk.tile([BS, E], F32, tag="sq")
            nc.scalar.activation(e_scr[:], c0[:], A.Exp, scale=-1.0, bias=m0[:],
                                 accum_out=pk[:, 0:1])
            e_scr2 = wk.tile([BS, E], F32, tag="sq")
            nc.scalar.activation(e_scr2[:], c1[:], A.Exp, scale=-1.0, bias=m1[:],
                                 accum_out=pk[:, 1:2])
            lnpk = sm.tile([BS, 2], F32, tag="s4")
            nc.scalar.activation(lnpk[:], pk[:], A.Ln)
            ia = sm.tile([BS, 1], F32, tag="s5")
            nc.vector.tensor_tensor(ia[:], lnpk[:, 1:2], lnpk[:, 0:1], ALU.subtract)
            ib = sm.tile([BS, 1], F32, tag="s6")
            nc.vector.tensor_tensor(ib[:], m0[:], m1[:], ALU.subtract)
            Dt = cst.tile([BS, 1], F32)
            nc.vector.tensor_scalar(Dt[:], ia[:], ib[:], LOGR, ALU.add, ALU.add)

            # ---------------- vanilla classifier (log_softmax) ----------------
            pv = pcls.tile([BS, C], F32, tag="cls")
            nc.tensor.matmul(pv[:], ones1[:], vb1[:], start=True, stop=False)
            for dc in range(4):
                nc.tensor.matmul(pv[:], xt[:, dc, :],
                                 vw[:, dc, :],
                                 start=False, stop=(dc == 3))
            vmax = sm.tile([BS, 1], F32, tag="s1")
            nc.vector.tensor_reduce(vmax[:], pv[:], mybir.AxisListType.X, ALU.max)
            nvmax = sm.tile([BS, 1], F32, tag="s2")
            nc.vector.tensor_scalar(nvmax[:], vmax[:], -1.0, None, ALU.mult)
            ve_scr = wk.tile([BS, C], F32, tag="sq")
            sv = sm.tile([BS, 1], F32, tag="s5")
            nc.scalar.activation(ve_scr[:], pv[:], A.Exp, bias=nvmax[:],
                                 accum_out=sv[:])
            lnsv = sm.tile([BS, 1], F32, tag="s6")
            nc.scalar.activation(lnsv[:], sv[:], A.Ln)
            van = wk.tile([BS, C], F32, tag="van")
            nc.vector.tensor_scalar(van[:], pv[:], nvmax[:], lnsv[:],
                                    ALU.add, ALU.subtract)
            nc.sync.dma_start(o_van[:, :], van[:])

            # ---------------- tanh classifier matmuls (tanh applied later) ----
            pt = pcls.tile([BS, C], F32, tag="cls")
            nc.tensor.matmul(pt[:], ones1[:], tb1[:], start=True, stop=False)
            for dc in range(4):
                nc.tensor.matmul(pt[:], xt[:, dc, :],
                                 tw[:, dc, :],
                                 start=False, stop=(dc == 3))

            # ---------------- expert logits: stream weights, matmul, drain ----
            # logits store: [BS, E*C] fp32 (128 KB per partition)
            lstore = stp.tile([BS, E * C], F32)
            for eh in range(E // 2):           # two experts per PSUM bank
                pe_t = pep.tile([BS, 2 * C], F32)
                for sub in range(2):
                    e_idx = 2 * eh + sub
                    ew_t = ewp.tile([128, 4, C], F32)
                    nc.sync.dma_start(
                        ew_t[:],
                        i_ewT.ap()[e_idx].rearrange("(a p) c -> p a c", p=128),
                    )
                    dst = pe_t[:, sub * C:(sub + 1) * C]
                    if with_expert_bias:
                        ebrow = ewp.tile([1, C], F32, tag="ebrow")
                        nc.sync.dma_start(ebrow[:], i_eb[e_idx:e_idx + 1, :])
                        nc.tensor.matmul(dst, ones1[:], ebrow[:],
                                         start=True, stop=False)
                    for dc in range(4):
                        nc.tensor.matmul(dst, xt[:, dc, :],
                                         ew_t[:, dc, :],
                                         start=(dc == 0 and not with_expert_bias),
                                         stop=(dc == 3))
                nc.scalar.activation(lstore[:, eh * 2 * C:(eh + 1) * 2 * C],
                                     pe_t[:], A.Copy)

            # ---------------- sinkhorn: 400 iterations ----------------
            sig_scr = wk.tile([BS, E], F32, tag="sig")
            pp = sm.tile([BS, 1], F32, tag="pp")
            ppi = sm.tile([BS, 1], I32, tag="ppi")
            uu = sm.tile([BS, 1], F32, tag="uu")
            hp = sm.tile([BS, 1], F32, tag="hp")
            ef = sm.tile([BS, 1], F32, tag="ef")
            Pt = sm.tile([BS, 1], F32, tag="Pt")
            for it in range(ITERS - 1):
                nc.scalar.activation(sig_scr[:], zt[:], A.Sigmoid, bias=Dt[:],
                                     accum_out=Pt[:])
                # y = N/P - 1
                nc.vector.reciprocal(pp[:], Pt[:])
                nc.vector.tensor_scalar(pp[:], pp[:], float(N), 1.0,
                                        ALU.mult, ALU.subtract)
                # software ln(y): exponent + mantissa poly
                nc.vector.tensor_scalar(ppi[:], pp[:].bitcast(I32), 23, None,
                                        ALU.arith_shift_right)
                nc.vector.tensor_scalar(uu[:].bitcast(I32), pp[:].bitcast(I32),
                                        0x007FFFFF, 0x3F800000,
                                        ALU.bitwise_and, ALU.bitwise_or)
                nc.vector.tensor_scalar(uu[:], uu[:], 1.0, None, ALU.subtract)
                nc.vector.tensor_scalar(hp[:], uu[:], LNC[0], LNC[1],
                                        ALU.mult, ALU.add)
                for ci in LNC[2:]:
                    nc.vector.tensor_scalar(hp[:], hp[:], uu[:], ci,
                                            ALU.mult, ALU.add)
                nc.vector.tensor_copy(ef[:], ppi[:])
                nc.vector.tensor_scalar(ef[:], ef[:], LN2, hp[:],
                                        ALU.mult, ALU.add)
                nc.vector.tensor_scalar(Dt[:], ef[:], LOGR, Dt[:],
                                        ALU.add, ALU.add)
            knn = cst.tile([BS, E], F32)
            nc.scalar.activation(knn[:], zt[:], A.Sigmoid, bias=Dt[:],
                                 accum_out=Pt[:])
            nc.sync.dma_start(o_knn[:, :], knn[:])

            # ---------------- tanh phase (same ACT table as sigmoid) --------
            tres = wk.tile([BS, C], F32, tag="van")
            nc.scalar.activation(tres[:], pt[:], A.Tanh, scale=1.0 / 10.0)
            nc.vector.tensor_scalar(tres[:], tres[:], 10.0, None, ALU.mult)
            nc.sync.dma_start(o_tanh[:, :], tres[:])
            for blk in range(E * C // 4096):
                sl = lstore[:, blk * 4096:(blk + 1) * 4096]
                nc.scalar.activation(sl, sl, A.Tanh, scale=1.0 / 10.0)

            # ---------------- gating + ensemble ----------------
            kk = wk.tile([BS, E], F32, tag="sig")
            nc.vector.tensor_tensor(kk[:], knn[:], knn[:], ALU.mult)
            sn = cst.tile([BS, E], F32)
            nc.vector.tensor_tensor(sn[:], kk[:], cos[:], ALU.mult)
            dnum = wk.tile([BS, E], F32, tag="sig")
            nc.vector.tensor_tensor(dnum[:], knn[:], cos[:], ALU.mult)
            den = sm.tile([BS, 1], F32, tag="s1")
            nc.vector.tensor_reduce(den[:], dnum[:], mybir.AxisListType.X, ALU.add)
            rden = sm.tile([BS, 1], F32, tag="s2")
            nc.vector.reciprocal(rden[:], den[:])

            pens = pnp.tile([BS, C], F32)
            for e_idx in range(E):
                dg = dgp.tile([BS, BS], F32)
                nc.vector.tensor_scalar(dg[:], iden[:], sn[:, e_idx:e_idx + 1],
                                        None, ALU.mult)
                nc.tensor.matmul(pens[:], dg[:],
                                 lstore[:, e_idx * C:(e_idx + 1) * C],
                                 start=(e_idx == 0), stop=(e_idx == E - 1))
            ens = wk.tile([BS, C], F32, tag="van")
            nc.scalar.activation(ens[:], pens[:], A.Copy, scale=rden[:])
            nc.sync.dma_start(o_ens[:, :], ens[:])

    nc.compile()
    return nc


_NC_CACHE = {}


def _get_nc(with_expert_bias):
    if with_expert_bias not in _NC_CACHE:
        _NC_CACHE[with_expert_bias] = build_nc(with_expert_bias)
    return _NC_CACHE[with_expert_bias]


def kernel(x, keys, expert_w, expert_b, vanilla_w, vanilla_b, tanh_w, tanh_b):
    global LAST_RESULTS
    from concourse.bass_utils import run_bass_kernel_spmd

    f32 = np.float32
    x = np.ascontiguousarray(x, dtype=f32)
    keysT = np.ascontiguousarray(np.asarray(keys, dtype=f32).T)
    ewT = np.ascontiguousarray(np.asarray(expert_w, dtype=f32).transpose(0, 2, 1))
    eb = np.ascontiguousarray(expert_b, dtype=f32)
    vwT = np.ascontiguousarray(np.asarray(vanilla_w, dtype=f32).T)
    vb = np.ascontiguousarray(vanilla_b, dtype=f32).reshape(1, C)
    twT = np.ascontiguousarray(np.asarray(tanh_w, dtype=f32).T)
    tb = np.ascontiguousarray(tanh_b, dtype=f32).reshape(1, C)
    iden10 = np.ascontiguousarray(np.eye(BS, dtype=f32) * f32(10.0))

    in_maps = []
    for i in range(NCORES):
        xs = np.ascontiguousarray(x[i * BS:(i + 1) * BS])
        in_maps.append({
            "xT": np.ascontiguousarray(xs.T),
            "x": xs,
            "keysT": keysT,
            "ewT": ewT,
            "eb": eb,
            "vwT": vwT,
            "vb": vb,
            "twT": twT,
            "tb": tb,
            "iden10": iden10,
        })

    nc = _get_nc(bool(np.any(eb)))
    trace = bool(int(os.environ.get("KERNEL_TRACE", "0")))
    res = run_bass_kernel_spmd(nc, in_maps, core_ids=list(range(NCORES)),
                               trace=trace)
    LAST_RESULTS = res

    ens = np.concatenate([res.results[i]["o_ens"] for i in range(NCORES)], axis=0)
    tout = np.concatenate([res.results[i]["o_tanh"] for i in range(NCORES)], axis=0)
    van = np.concatenate([res.results[i]["o_van"] for i in range(NCORES)], axis=0)
    cdist = np.concatenate([res.results[i]["o_cd"] for i in range(NCORES)], axis=0)
    knn = np.concatenate([res.results[i]["o_knn"] for i in range(NCORES)], axis=0)
    return (ens, tout, van, cdist, knn)


# revision 20
# speedup vs baseline: 1.1166x; 1.1166x over previous
"""Trainium2 Bass kernel for nn_EnsembleE2EModule (moe_routing).

Strategy: fully data-parallel over batch B=1024 across 8 cores (128 samples
per core). Each core computes, for its batch shard:
  - cos_sim / cos_dist against all 128 keys (fp32 matmul)
  - the 400-iteration Sinkhorn soft-top-k, reduced EXACTLY to a scalar
    per-sample recursion:  P = sum_i sigmoid(Delta + z_i)  (one ACT
    sigmoid+accumulate op per iteration), then
    Delta' = Delta + log(k/(N-k)) + ln(N/P - 1)  with the ln computed on the
    vector engine in software (exponent/mantissa split + deg-6 polynomial).
    knn_sim = sigmoid(Delta_400 + z) elementwise (the final sigmoid output).
  - all 128 experts' logits x @ W_e^T via fp32r matmuls (weights streamed
    from HBM pre-transposed on host), drained PSUM->SBUF, tanh'd, then gated
    and summed over experts with diagonal-matmul accumulation in PSUM.
  - vanilla (log_softmax) and tanh classifiers.
Host side only shards/transposes inputs and concatenates per-core outputs.
"""

import os
import sys

import numpy as np

sys.path.insert(0, "/opt/trn_rl_repo")

import concourse.bass as bass  # noqa: E402
import concourse.bacc as bacc  # noqa: E402
import concourse.mybir as mybir  # noqa: E402
import concourse.tile as tile  # noqa: E402

A = mybir.ActivationFunctionType
ALU = mybir.AluOpType
F32 = mybir.dt.float32
F32R = mybir.dt.float32r
I32 = mybir.dt.int32

B, D, E, C = 1024, 512, 128, 256
NCORES = 8
BS = B // NCORES            # 128 samples per core
N = E                       # sinkhorn column count
K = 16
EPS = 5e-4
ITERS = 400
SQ = float(np.float32(np.sqrt(1.0 / EPS)))        # 44.72136
LOGR = float(np.float32(np.log(K / (N - K))))     # ln(16/112)
LN2 = float(np.float32(np.log(2.0)))

# deg-5 polynomial for ln(m) on [1,2), highest degree first (~2.2e-5 abs err)
_m = np.linspace(1, 2, 20001)
_P5 = np.polynomial.chebyshev.Chebyshev.fit(_m, np.log(_m), 5).convert(
    kind=np.polynomial.Polynomial
)
LNC = [float(np.float32(v)) for v in _P5.coef[::-1]]
# raw (biased) exponent is used in the final fma; fold -127*ln2 into the
# polynomial's constant term so the exponent op needs no arithmetic fixup
LNC[-1] = float(np.float32(LNC[-1] - 127.0 * np.log(2.0)))

LAST_RESULTS = None


def build_nc(with_expert_bias=False):
    nc = bacc.Bacc(
        "TRN2", target_bir_lowering=False, debug=False, enable_asserts=False,
        num_devices=NCORES,
    )

    # ---- I/O ----
    i_xT = nc.dram_tensor("xT", [D, BS], F32, kind="ExternalInput")
    i_x = nc.dram_tensor("x", [BS, D], F32, kind="ExternalInput")
    i_kT = nc.dram_tensor("keysT", [D, E], F32, kind="ExternalInput")
    i_ewT = nc.dram_tensor("ewT", [E, D, C], F32, kind="ExternalInput")
    i_eb = nc.dram_tensor("eb", [E, C], F32, kind="ExternalInput")
    i_vwT = nc.dram_tensor("vwT", [D, C], F32, kind="ExternalInput")
    i_vb = nc.dram_tensor("vb", [1, C], F32, kind="ExternalInput")
    i_twT = nc.dram_tensor("twT", [D, C], F32, kind="ExternalInput")
    i_tb = nc.dram_tensor("tb", [1, C], F32, kind="ExternalInput")
    i_id = nc.dram_tensor("iden10", [BS, BS], F32, kind="ExternalInput")

    o_ens = nc.dram_tensor("o_ens", [BS, C], F32, kind="ExternalOutput")
    o_tanh = nc.dram_tensor("o_tanh", [BS, C], F32, kind="ExternalOutput")
    o_van = nc.dram_tensor("o_van", [BS, C], F32, kind="ExternalOutput")
    o_cd = nc.dram_tensor("o_cd", [BS, N], F32, kind="ExternalOutput")
    o_knn = nc.dram_tensor("o_knn", [BS, N], F32, kind="ExternalOutput")

    with tile.TileContext(nc) as tc:
        with (
            tc.tile_pool(name="consts", bufs=1) as cst,
            tc.tile_pool(name="ew", bufs=4) as ewp,
            tc.tile_pool(name="store", bufs=1) as stp,
            tc.tile_pool(name="work", bufs=2) as wk,
            tc.tile_pool(name="small", bufs=2) as sm,
            tc.tile_pool(name="diagp", bufs=3) as dgp,
            tc.tile_pool(name="pc", bufs=1, space="PSUM") as ppc,
            tc.tile_pool(name="pcls", bufs=2, space="PSUM") as pcls,
            tc.tile_pool(name="pe", bufs=3, space="PSUM") as pep,
            tc.tile_pool(name="psink", bufs=1, space="PSUM") as psk,
        ):
            # ---------------- constant loads ----------------
            xt = cst.tile([128, 4, BS], F32)       # xT as 4 d-chunks
            nc.sync.dma_start(xt[:], i_xT.ap().rearrange("(a p) c -> p a c", p=128))
            xn = cst.tile([BS, D], F32)
            nc.sync.dma_start(xn[:], i_x[:, :])
            kt = cst.tile([128, 4, E], F32)
            nc.sync.dma_start(kt[:], i_kT.ap().rearrange("(a p) c -> p a c", p=128))
            vw = cst.tile([128, 4, C], F32)
            nc.sync.dma_start(vw[:], i_vwT.ap().rearrange("(a p) c -> p a c", p=128))
            tw = cst.tile([128, 4, C], F32)
            nc.sync.dma_start(tw[:], i_twT.ap().rearrange("(a p) c -> p a c", p=128))
            vb1 = cst.tile([1, C], F32)
            nc.sync.dma_start(vb1[:], i_vb[:, :])
            tb1 = cst.tile([1, C], F32)
            nc.sync.dma_start(tb1[:], i_tb[:, :])
            iden = cst.tile([BS, BS], F32)
            nc.sync.dma_start(iden[:], i_id[:, :])
            ones1 = cst.tile([1, BS], F32)
            nc.vector.memset(ones1[:], 1.0)

            # ---------------- row norms of x ----------------
            sq_scr = wk.tile([BS, D], F32, tag="sq")
            sqn = cst.tile([BS, 1], F32)
            nc.scalar.activation(sq_scr[:], xn[:], A.Square, accum_out=sqn[:])
            ln_s = cst.tile([BS, 1], F32)
            nc.scalar.activation(ln_s[:], sqn[:], A.Ln)
            rnorm = cst.tile([BS, 1], F32)
            nc.scalar.activation(rnorm[:], ln_s[:], A.Exp, scale=-0.5)

            # ---------------- cos_sim / cos_dist / z ----------------
            pcos = ppc.tile([BS, E], F32)
            for dc in range(4):
                nc.tensor.matmul(pcos[:], xt[:, dc, :], kt[:, dc, :],
                                 start=(dc == 0), stop=(dc == 3))
            cos = cst.tile([BS, E], F32)
            nc.scalar.activation(cos[:], pcos[:], A.Copy, scale=rnorm[:])
            cd = cst.tile([BS, E], F32)
            nc.vector.tensor_scalar(cd[:], cos[:], -1.0, 1.0, ALU.mult, ALU.add)
            nc.sync.dma_start(o_cd[:, :], cd[:])
            rowmax = sm.tile([BS, 1], F32, tag="s1")
            nc.vector.tensor_reduce(rowmax[:], cd[:], mybir.AxisListType.X, ALU.max)
            invmax = sm.tile([BS, 1], F32, tag="s2")
            nc.vector.reciprocal(invmax[:], rowmax[:])
            dh = cst.tile([BS, E], F32)
            nc.vector.tensor_scalar(dh[:], cd[:], invmax[:], SQ, ALU.mult, ALU.mult)
            c0 = cst.tile([BS, E], F32)
            nc.scalar.activation(c0[:], dh[:], A.Square)
            dh2 = cst.tile([BS, E], F32)
            nc.vector.tensor_scalar(dh2[:], dh[:], SQ, None, ALU.subtract)
            c1 = cst.tile([BS, E], F32)
            nc.scalar.activation(c1[:], dh2[:], A.Square)
            zt = psk.tile([BS, E], F32, tag="zt")
            nc.vector.tensor_tensor(zt[:], c1[:], c0[:], ALU.subtract)

            # ---------------- sinkhorn init: Delta_1 ----------------
            m0 = sm.tile([BS, 1], F32, tag="s1")
            nc.vector.tensor_reduce(m0[:], c0[:], mybir.AxisListType.X, ALU.min)
            m1 = sm.tile([BS, 1], F32, tag="s2")
            nc.vector.tensor_reduce(m1[:], c1[:], mybir.AxisListType.X, ALU.min)
            pk = sm.tile([BS, 2], F32, tag="s3")
            e_scr = wk.tile([BS, E], F32, tag="sq")
            nc.scalar.activation(e_scr[:], c0[:], A.Exp, scale=-1.0, bias=m0[:],
                                 accum_out=pk[:, 0:1])
            e_scr2 = wk.tile([BS, E], F32, tag="sq")
            nc.scalar.activation(e_scr2[:], c1[:], A.Exp, scale=-1.0, bias=m1[:],
                                 accum_out=pk[:, 1:2])
            lnpk = sm.tile([BS, 2], F32, tag="s4")
            nc.scalar.activation(lnpk[:], pk[:], A.Ln)
            ia = sm.tile([BS, 1], F32, tag="s5")
            nc.vector.tensor_tensor(ia[:], lnpk[:, 1:2], lnpk[:, 0:1], ALU.subtract)
            ib = sm.tile([BS, 1], F32, tag="s6")
            nc.vector.tensor_tensor(ib[:], m0[:], m1[:], ALU.subtract)
            Dt = cst.tile([BS, 1], F32)
            nc.vector.tensor_scalar(Dt[:], ia[:], ib[:], LOGR, ALU.add, ALU.add)

            # ---------------- vanilla classifier (log_softmax) ----------------
            pv = pcls.tile([BS, C], F32, tag="cls")
            nc.tensor.matmul(pv[:], ones1[:], vb1[:], start=True, stop=False)
            for dc in range(4):
                nc.tensor.matmul(pv[:], xt[:, dc, :],
                                 vw[:, dc, :],
                                 start=False, stop=(dc == 3))
            vmax = sm.tile([BS, 1], F32, tag="s1")
            nc.vector.tensor_reduce(vmax[:], pv[:], mybir.AxisListType.X, ALU.max)
            nvmax = sm.tile([BS, 1], F32, tag="s2")
            nc.vector.tensor_scalar(nvmax[:], vmax[:], -1.0, None, ALU.mult)
            ve_scr = wk.tile([BS, C], F32, tag="sq")
            sv = sm.tile([BS, 1], F32, tag="s5")
            nc.scalar.activation(ve_scr[:], pv[:], A.Exp, bias=nvmax[:],
                                 accum_out=sv[:])
            lnsv = sm.tile([BS, 1], F32, tag="s6")
            nc.scalar.activation(lnsv[:], sv[:], A.Ln)
            van = wk.tile([BS, C], F32, tag="van")
            nc.vector.tensor_scalar(van[:], pv[:], nvmax[:], lnsv[:],
                                    ALU.add, ALU.subtract)
            nc.sync.dma_start(o_van[:, :], van[:])

            # ---------------- tanh classifier matmuls (tanh applied later) ----
            pt = pcls.tile([BS, C], F32, tag="cls")
            nc.tensor.matmul(pt[:], ones1[:], tb1[:], start=True, stop=False)
            for dc in range(4):
                nc.tensor.matmul(pt[:], xt[:, dc, :],
                                 tw[:, dc, :],
                                 start=False, stop=(dc == 3))

            # ---------------- expert logits: stream weights, matmul, drain ----
            # logits store: [BS, E*C] fp32 (128 KB per partition)
            lstore = stp.tile([BS, E * C], F32)
            for eh in range(E // 2):           # two experts per PSUM bank
                pe_t = pep.tile([BS, 2 * C], F32)
                for sub in range(2):
                    e_idx = 2 * eh + sub
                    ew_t = ewp.tile([128, 4, C], F32)
                    nc.sync.dma_start(
                        ew_t[:],
                        i_ewT.ap()[e_idx].rearrange("(a p) c -> p a c", p=128),
                    )
                    dst = pe_t[:, sub * C:(sub + 1) * C]
                    if with_expert_bias:
                        ebrow = ewp.tile([1, C], F32, tag="ebrow")
                        nc.sync.dma_start(ebrow[:], i_eb[e_idx:e_idx + 1, :])
                        nc.tensor.matmul(dst, ones1[:], ebrow[:],
                                         start=True, stop=False)
                    for dc in range(4):
                        nc.tensor.matmul(dst, xt[:, dc, :],
                                         ew_t[:, dc, :],
                                         start=(dc == 0 and not with_expert_bias),
                                         stop=(dc == 3))
                nc.scalar.activation(lstore[:, eh * 2 * C:(eh + 1) * 2 * C],
                                     pe_t[:], A.Copy)

            # ---------------- sinkhorn: 400 iterations ----------------
            sig_scr = psk.tile([BS, E], F32, tag="sigp")
            pp = sm.tile([BS, 1], F32, tag="pp")
            ppi = sm.tile([BS, 1], I32, tag="ppi")
            uu = sm.tile([BS, 1], F32, tag="uu")
            hp = sm.tile([BS, 1], F32, tag="hp")
            ef = sm.tile([BS, 1], F32, tag="ef")
            Pt = sm.tile([BS, 1], F32, tag="Pt")
            for it in range(ITERS - 1):
                nc.scalar.activation(sig_scr[:], zt[:], A.Sigmoid, bias=Dt[:],
                                     accum_out=Pt[:])
                # y = N/P - 1
                nc.vector.reciprocal(pp[:], Pt[:])
                nc.vector.tensor_scalar(pp[:], pp[:], float(N), 1.0,
                                        ALU.mult, ALU.subtract)
                # software ln(y): exponent + mantissa poly
                nc.vector.tensor_scalar(ppi[:], pp[:].bitcast(I32), 23, None,
                                        ALU.arith_shift_right)
                nc.vector.tensor_scalar(uu[:].bitcast(I32), pp[:].bitcast(I32),
                                        0x007FFFFF, 0x3F800000,
                                        ALU.bitwise_and, ALU.bitwise_or)
                nc.vector.tensor_scalar(hp[:], uu[:], LNC[0], LNC[1],
                                        ALU.mult, ALU.add)
                for ci in LNC[2:]:
                    nc.vector.tensor_scalar(hp[:], hp[:], uu[:], ci,
                                            ALU.mult, ALU.add)
                nc.vector.tensor_copy(ef[:], ppi[:])
                nc.vector.tensor_scalar(ef[:], ef[:], LN2, hp[:],
                                        ALU.mult, ALU.add)
                nc.vector.tensor_scalar(Dt[:], ef[:], LOGR, Dt[:],
                                        ALU.add, ALU.add)
            knn = cst.tile([BS, E], F32)
            nc.scalar.activation(knn[:], zt[:], A.Sigmoid, bias=Dt[:],
                                 accum_out=Pt[:])
            nc.sync.dma_start(o_knn[:, :], knn[:])

            # ---------------- tanh phase (same ACT table as sigmoid) --------
            tres = wk.tile([BS, C], F32, tag="van")
            nc.scalar.activation(tres[:], pt[:], A.Tanh, scale=1.0 / 10.0)
            nc.vector.tensor_scalar(tres[:], tres[:], 10.0, None, ALU.mult)
            nc.sync.dma_start(o_tanh[:, :], tres[:])
            for blk in range(E * C // 4096):
                sl = lstore[:, blk * 4096:(blk + 1) * 4096]
                nc.scalar.activation(sl, sl, A.Tanh, scale=1.0 / 10.0)

            # ---------------- gating + ensemble ----------------
            kk = wk.tile([BS, E], F32, tag="sig")
            nc.vector.tensor_tensor(kk[:], knn[:], knn[:], ALU.mult)
            sn = cst.tile([BS, E], F32)
            nc.vector.tensor_tensor(sn[:], kk[:], cos[:], ALU.mult)
            dnum = wk.tile([BS, E], F32, tag="sig")
            nc.vector.tensor_tensor(dnum[:], knn[:], cos[:], ALU.mult)
            den = sm.tile([BS, 1], F32, tag="s1")
            nc.vector.tensor_reduce(den[:], dnum[:], mybir.AxisListType.X, ALU.add)
            rden = sm.tile([BS, 1], F32, tag="s2")
            nc.vector.reciprocal(rden[:], den[:])

            pens = ppc.tile([BS, C], F32, tag="pcos")
            for e_idx in range(E):
                dg = dgp.tile([BS, BS], F32)
                nc.vector.tensor_scalar(dg[:], iden[:], sn[:, e_idx:e_idx + 1],
                                        None, ALU.mult)
                nc.tensor.matmul(pens[:], dg[:],
                                 lstore[:, e_idx * C:(e_idx + 1) * C],
                                 start=(e_idx == 0), stop=(e_idx == E - 1))
            ens = wk.tile([BS, C], F32, tag="van")
            nc.scalar.activation(ens[:], pens[:], A.Copy, scale=rden[:])
            nc.sync.dma_start(o_ens[:, :], ens[:])

    nc.compile()
    return nc


_NC_CACHE = {}


def _get_nc(with_expert_bias):
    if with_expert_bias not in _NC_CACHE:
        _NC_CACHE[with_expert_bias] = build_nc(with_expert_bias)
    return _NC_CACHE[with_expert_bias]


def kernel(x, keys, expert_w, expert_b, vanilla_w, vanilla_b, tanh_w, tanh_b):
    global LAST_RESULTS
    from concourse.bass_utils import run_bass_kernel_spmd

    f32 = np.float32
    x = np.ascontiguousarray(x, dtype=f32)
    keysT = np.ascontiguousarray(np.asarray(keys, dtype=f32).T)
    ewT = np.ascontiguousarray(np.asarray(expert_w, dtype=f32).transpose(0, 2, 1))
    eb = np.ascontiguousarray(expert_b, dtype=f32)
    vwT = np.ascontiguousarray(np.asarray(vanilla_w, dtype=f32).T)
    vb = np.ascontiguousarray(vanilla_b, dtype=f32).reshape(1, C)
    twT = np.ascontiguousarray(np.asarray(tanh_w, dtype=f32).T)
    tb = np.ascontiguousarray(tanh_b, dtype=f32).reshape(1, C)
    iden10 = np.ascontiguousarray(np.eye(BS, dtype=f32) * f32(10.0))

    in_maps = []
    for i in range(NCORES):
        xs = np.ascontiguousarray(x[i * BS:(i + 1) * BS])
        in_maps.append({
            "xT": np.ascontiguousarray(xs.T),
            "x": xs,
            "keysT": keysT,
            "ewT": ewT,
            "eb": eb,
            "vwT": vwT,
            "vb": vb,
            "twT": twT,
            "tb": tb,
            "iden10": iden10,
        })

    nc = _get_nc(bool(np.any(eb)))
    trace = bool(int(os.environ.get("KERNEL_TRACE", "0")))
    res = run_bass_kernel_spmd(nc, in_maps, core_ids=list(range(NCORES)),
                               trace=trace)
    LAST_RESULTS = res

    ens = np.concatenate([res.results[i]["o_ens"] for i in range(NCORES)], axis=0)
    tout = np.concatenate([res.results[i]["o_tanh"] for i in range(NCORES)], axis=0)
    van = np.concatenate([res.results[i]["o_van"] for i in range(NCORES)], axis=0)
    cdist = np.concatenate([res.results[i]["o_cd"] for i in range(NCORES)], axis=0)
    knn = np.concatenate([res.results[i]["o_knn"] for i in range(NCORES)], axis=0)
    return (ens, tout, van, cdist, knn)


# revision 23
# speedup vs baseline: 1.1912x; 1.0668x over previous
"""Trainium2 Bass kernel for nn_EnsembleE2EModule (moe_routing).

Strategy: fully data-parallel over batch B=1024 across 8 cores (128 samples
per core). Each core computes, for its batch shard:
  - cos_sim / cos_dist against all 128 keys (fp32 matmul)
  - the 400-iteration Sinkhorn soft-top-k, reduced EXACTLY to a scalar
    per-sample recursion:  P = sum_i sigmoid(Delta + z_i)  (one ACT
    sigmoid+accumulate op per iteration), then
    Delta' = Delta + log(k/(N-k)) + ln(N/P - 1)  with the ln computed on the
    vector engine in software (exponent/mantissa split + deg-6 polynomial).
    knn_sim = sigmoid(Delta_400 + z) elementwise (the final sigmoid output).
  - all 128 experts' logits x @ W_e^T via fp32 matmuls (weights streamed
    from HBM pre-transposed on host), drained PSUM->SBUF, tanh'd, then gated
    and summed over experts with diagonal-matmul accumulation in PSUM.
  - vanilla (log_softmax) and tanh classifiers.
Host side only shards/transposes inputs and concatenates per-core outputs.
"""

import os
import sys

import numpy as np

sys.path.insert(0, "/opt/trn_rl_repo")

import concourse.bass as bass  # noqa: E402
import concourse.bacc as bacc  # noqa: E402
import concourse.mybir as mybir  # noqa: E402
import concourse.tile as tile  # noqa: E402

A = mybir.ActivationFunctionType
ALU = mybir.AluOpType
F32 = mybir.dt.float32
F32R = mybir.dt.float32r
I32 = mybir.dt.int32

B, D, E, C = 1024, 512, 128, 256
NCORES = 8
BS = B // NCORES            # 128 samples per core
N = E                       # sinkhorn column count
K = 16
EPS = 5e-4
ITERS = 400
SQ = float(np.float32(np.sqrt(1.0 / EPS)))        # 44.72136
LOGR = float(np.float32(np.log(K / (N - K))))     # ln(16/112)
LN2 = float(np.float32(np.log(2.0)))

# deg-5 polynomial for ln(m) on [1,2), highest degree first (~2.2e-5 abs err)
_m = np.linspace(1, 2, 20001)
_P5 = np.polynomial.chebyshev.Chebyshev.fit(_m, np.log(_m), 5).convert(
    kind=np.polynomial.Polynomial
)
LNC = [float(np.float32(v)) for v in _P5.coef[::-1]]
# raw (biased) exponent is used in the final fma; fold -127*ln2 into the
# polynomial's constant term so the exponent op needs no arithmetic fixup
LNC[-1] = float(np.float32(LNC[-1] - 127.0 * np.log(2.0)))

LAST_RESULTS = None


def build_nc(with_expert_bias=False):
    nc = bacc.Bacc(
        "TRN2", target_bir_lowering=False, debug=False, enable_asserts=False,
        num_devices=NCORES,
    )

    # ---- I/O ----
    i_xT = nc.dram_tensor("xT", [D, BS], F32, kind="ExternalInput")
    i_x = nc.dram_tensor("x", [BS, D], F32, kind="ExternalInput")
    i_kT = nc.dram_tensor("keysT", [D, E], F32, kind="ExternalInput")
    i_ewT = nc.dram_tensor("ewT", [E, D, C], F32, kind="ExternalInput")
    i_eb = nc.dram_tensor("eb", [E, C], F32, kind="ExternalInput")
    i_vwT = nc.dram_tensor("vwT", [D, C], F32, kind="ExternalInput")
    i_vb = nc.dram_tensor("vb", [1, C], F32, kind="ExternalInput")
    i_twT = nc.dram_tensor("twT", [D, C], F32, kind="ExternalInput")
    i_tb = nc.dram_tensor("tb", [1, C], F32, kind="ExternalInput")
    i_id = nc.dram_tensor("iden10", [BS, BS], F32, kind="ExternalInput")

    o_ens = nc.dram_tensor("o_ens", [BS, C], F32, kind="ExternalOutput")
    o_tanh = nc.dram_tensor("o_tanh", [BS, C], F32, kind="ExternalOutput")
    o_van = nc.dram_tensor("o_van", [BS, C], F32, kind="ExternalOutput")
    o_cd = nc.dram_tensor("o_cd", [BS, N], F32, kind="ExternalOutput")
    o_knn = nc.dram_tensor("o_knn", [BS, N], F32, kind="ExternalOutput")

    with tile.TileContext(nc) as tc:
        with (
            tc.tile_pool(name="consts", bufs=1) as cst,
            tc.tile_pool(name="ew", bufs=4) as ewp,
            tc.tile_pool(name="store", bufs=1) as stp,
            tc.tile_pool(name="work", bufs=2) as wk,
            tc.tile_pool(name="small", bufs=2) as sm,
            tc.tile_pool(name="diagp", bufs=3) as dgp,
            tc.tile_pool(name="pc", bufs=1, space="PSUM") as ppc,
            tc.tile_pool(name="pcls", bufs=2, space="PSUM") as pcls,
            tc.tile_pool(name="pe", bufs=3, space="PSUM") as pep,
            tc.tile_pool(name="psink", bufs=1, space="PSUM") as psk,
        ):
            # ---------------- constant loads ----------------
            xt = cst.tile([128, 4, BS], F32)       # xT as 4 d-chunks
            nc.sync.dma_start(xt[:], i_xT.ap().rearrange("(a p) c -> p a c", p=128))
            xn = cst.tile([BS, D], F32)
            nc.sync.dma_start(xn[:], i_x[:, :])
            kt = cst.tile([128, 4, E], F32)
            nc.sync.dma_start(kt[:], i_kT.ap().rearrange("(a p) c -> p a c", p=128))
            vw = cst.tile([128, 4, C], F32)
            nc.sync.dma_start(vw[:], i_vwT.ap().rearrange("(a p) c -> p a c", p=128))
            tw = cst.tile([128, 4, C], F32)
            nc.sync.dma_start(tw[:], i_twT.ap().rearrange("(a p) c -> p a c", p=128))
            vb1 = cst.tile([1, C], F32)
            nc.sync.dma_start(vb1[:], i_vb[:, :])
            tb1 = cst.tile([1, C], F32)
            nc.sync.dma_start(tb1[:], i_tb[:, :])
            iden = cst.tile([BS, BS], F32)
            nc.sync.dma_start(iden[:], i_id[:, :])
            ones1 = cst.tile([1, BS], F32)
            nc.vector.memset(ones1[:], 1.0)

            # ---------------- row norms of x ----------------
            sq_scr = wk.tile([BS, D], F32, tag="sq")
            sqn = cst.tile([BS, 1], F32)
            nc.scalar.activation(sq_scr[:], xn[:], A.Square, accum_out=sqn[:])
            ln_s = cst.tile([BS, 1], F32)
            nc.scalar.activation(ln_s[:], sqn[:], A.Ln)
            rnorm = cst.tile([BS, 1], F32)
            nc.scalar.activation(rnorm[:], ln_s[:], A.Exp, scale=-0.5)

            # ---------------- cos_sim / cos_dist / z ----------------
            pcos = ppc.tile([BS, E], F32)
            for dc in range(4):
                nc.tensor.matmul(pcos[:], xt[:, dc, :], kt[:, dc, :],
                                 start=(dc == 0), stop=(dc == 3))
            cos = cst.tile([BS, E], F32)
            nc.scalar.activation(cos[:], pcos[:], A.Copy, scale=rnorm[:])
            cd = cst.tile([BS, E], F32)
            nc.vector.tensor_scalar(cd[:], cos[:], -1.0, 1.0, ALU.mult, ALU.add)
            nc.sync.dma_start(o_cd[:, :], cd[:])
            rowmax = sm.tile([BS, 1], F32, tag="s1")
            nc.vector.tensor_reduce(rowmax[:], cd[:], mybir.AxisListType.X, ALU.max)
            invmax = sm.tile([BS, 1], F32, tag="s2")
            nc.vector.reciprocal(invmax[:], rowmax[:])
            dh = cst.tile([BS, E], F32)
            nc.vector.tensor_scalar(dh[:], cd[:], invmax[:], SQ, ALU.mult, ALU.mult)
            c0 = cst.tile([BS, E], F32)
            nc.scalar.activation(c0[:], dh[:], A.Square)
            dh2 = cst.tile([BS, E], F32)
            nc.vector.tensor_scalar(dh2[:], dh[:], SQ, None, ALU.subtract)
            c1 = cst.tile([BS, E], F32)
            nc.scalar.activation(c1[:], dh2[:], A.Square)
            zt = psk.tile([BS, E], F32, tag="zt")
            nc.vector.tensor_tensor(zt[:], c1[:], c0[:], ALU.subtract)

            # ---------------- sinkhorn init: Delta_1 ----------------
            m0 = sm.tile([BS, 1], F32, tag="s1")
            nc.vector.tensor_reduce(m0[:], c0[:], mybir.AxisListType.X, ALU.min)
            m1 = sm.tile([BS, 1], F32, tag="s2")
            nc.vector.tensor_reduce(m1[:], c1[:], mybir.AxisListType.X, ALU.min)
            pk = sm.tile([BS, 2], F32, tag="s3")
            e_scr = wk.tile([BS, E], F32, tag="sq")
            nc.scalar.activation(e_scr[:], c0[:], A.Exp, scale=-1.0, bias=m0[:],
                                 accum_out=pk[:, 0:1])
            e_scr2 = wk.tile([BS, E], F32, tag="sq")
            nc.scalar.activation(e_scr2[:], c1[:], A.Exp, scale=-1.0, bias=m1[:],
                                 accum_out=pk[:, 1:2])
            lnpk = sm.tile([BS, 2], F32, tag="s4")
            nc.scalar.activation(lnpk[:], pk[:], A.Ln)
            ia = sm.tile([BS, 1], F32, tag="s5")
            nc.vector.tensor_tensor(ia[:], lnpk[:, 1:2], lnpk[:, 0:1], ALU.subtract)
            ib = sm.tile([BS, 1], F32, tag="s6")
            nc.vector.tensor_tensor(ib[:], m0[:], m1[:], ALU.subtract)
            Dt = cst.tile([BS, 1], F32)
            nc.vector.tensor_scalar(Dt[:], ia[:], ib[:], LOGR, ALU.add, ALU.add)

            # ---------------- vanilla classifier (log_softmax) ----------------
            pv = pcls.tile([BS, C], F32, tag="cls")
            nc.tensor.matmul(pv[:], ones1[:], vb1[:], start=True, stop=False)
            for dc in range(4):
                nc.tensor.matmul(pv[:], xt[:, dc, :],
                                 vw[:, dc, :],
                                 start=False, stop=(dc == 3))
            vmax = sm.tile([BS, 1], F32, tag="s1")
            nc.vector.tensor_reduce(vmax[:], pv[:], mybir.AxisListType.X, ALU.max)
            nvmax = sm.tile([BS, 1], F32, tag="s2")
            nc.vector.tensor_scalar(nvmax[:], vmax[:], -1.0, None, ALU.mult)
            ve_scr = wk.tile([BS, C], F32, tag="sq")
            sv = sm.tile([BS, 1], F32, tag="s5")
            nc.scalar.activation(ve_scr[:], pv[:], A.Exp, bias=nvmax[:],
                                 accum_out=sv[:])
            lnsv = sm.tile([BS, 1], F32, tag="s6")
            nc.scalar.activation(lnsv[:], sv[:], A.Ln)
            van = wk.tile([BS, C], F32, tag="van")
            nc.vector.tensor_scalar(van[:], pv[:], nvmax[:], lnsv[:],
                                    ALU.add, ALU.subtract)
            nc.sync.dma_start(o_van[:, :], van[:])

            # ---------------- tanh classifier matmuls (tanh applied later) ----
            pt = pcls.tile([BS, C], F32, tag="cls")
            nc.tensor.matmul(pt[:], ones1[:], tb1[:], start=True, stop=False)
            for dc in range(4):
                nc.tensor.matmul(pt[:], xt[:, dc, :],
                                 tw[:, dc, :],
                                 start=False, stop=(dc == 3))

            # ---------------- expert logits: stream weights, matmul, drain ----
            # logits store: [BS, E*C] fp32 (128 KB per partition)
            lstore = stp.tile([BS, E * C], F32R)
            for eh in range(E // 2):           # two experts per PSUM bank
                pe_t = pep.tile([BS, 2 * C], F32)
                for sub in range(2):
                    e_idx = 2 * eh + sub
                    ew_t = ewp.tile([128, 4, C], F32)
                    nc.sync.dma_start(
                        ew_t[:],
                        i_ewT.ap()[e_idx].rearrange("(a p) c -> p a c", p=128),
                    )
                    dst = pe_t[:, sub * C:(sub + 1) * C]
                    if with_expert_bias:
                        ebrow = ewp.tile([1, C], F32, tag="ebrow")
                        nc.sync.dma_start(ebrow[:], i_eb[e_idx:e_idx + 1, :])
                        nc.tensor.matmul(dst, ones1[:], ebrow[:],
                                         start=True, stop=False)
                    for dc in range(4):
                        nc.tensor.matmul(dst, xt[:, dc, :],
                                         ew_t[:, dc, :],
                                         start=(dc == 0 and not with_expert_bias),
                                         stop=(dc == 3))
                nc.scalar.activation(lstore[:, eh * 2 * C:(eh + 1) * 2 * C],
                                     pe_t[:], A.Copy)

            # ---------------- sinkhorn: 400 iterations ----------------
            sig_scr = psk.tile([BS, E], F32, tag="sigp")
            pp = sm.tile([BS, 1], F32, tag="pp")
            ppi = sm.tile([BS, 1], I32, tag="ppi")
            uu = sm.tile([BS, 1], F32, tag="uu")
            hp = sm.tile([BS, 1], F32, tag="hp")
            ef = sm.tile([BS, 1], F32, tag="ef")
            Pt = sm.tile([BS, 1], F32, tag="Pt")
            for it in range(ITERS - 1):
                nc.scalar.activation(sig_scr[:], zt[:], A.Sigmoid, bias=Dt[:],
                                     accum_out=Pt[:])
                # y = N/P - 1
                nc.vector.reciprocal(pp[:], Pt[:])
                nc.vector.tensor_scalar(pp[:], pp[:], float(N), 1.0,
                                        ALU.mult, ALU.subtract)
                # software ln(y): exponent + mantissa poly
                nc.vector.tensor_scalar(ppi[:], pp[:].bitcast(I32), 23, None,
                                        ALU.arith_shift_right)
                nc.vector.tensor_scalar(uu[:].bitcast(I32), pp[:].bitcast(I32),
                                        0x007FFFFF, 0x3F800000,
                                        ALU.bitwise_and, ALU.bitwise_or)
                nc.vector.tensor_scalar(hp[:], uu[:], LNC[0], LNC[1],
                                        ALU.mult, ALU.add)
                for ci in LNC[2:]:
                    nc.vector.tensor_scalar(hp[:], hp[:], uu[:], ci,
                                            ALU.mult, ALU.add)
                nc.vector.tensor_copy(ef[:], ppi[:])
                nc.vector.tensor_scalar(ef[:], ef[:], LN2, hp[:],
                                        ALU.mult, ALU.add)
                nc.vector.tensor_scalar(Dt[:], ef[:], LOGR, Dt[:],
                                        ALU.add, ALU.add)
            knn = cst.tile([BS, E], F32)
            nc.scalar.activation(knn[:], zt[:], A.Sigmoid, bias=Dt[:],
                                 accum_out=Pt[:])
            nc.sync.dma_start(o_knn[:, :], knn[:])

            # ---------------- tanh phase (same ACT table as sigmoid) --------
            tres = wk.tile([BS, C], F32, tag="van")
            nc.scalar.activation(tres[:], pt[:], A.Tanh, scale=1.0 / 10.0)
            nc.vector.tensor_scalar(tres[:], tres[:], 10.0, None, ALU.mult)
            nc.sync.dma_start(o_tanh[:, :], tres[:])
            for blk in range(E * C // 4096):
                sl = lstore[:, blk * 4096:(blk + 1) * 4096]
                nc.scalar.activation(sl, sl, A.Tanh, scale=1.0 / 10.0)

            # ---------------- gating + ensemble ----------------
            kk = wk.tile([BS, E], F32, tag="sig")
            nc.vector.tensor_tensor(kk[:], knn[:], knn[:], ALU.mult)
            sn = cst.tile([BS, E], F32)
            nc.vector.tensor_tensor(sn[:], kk[:], cos[:], ALU.mult)
            dnum = wk.tile([BS, E], F32, tag="sig")
            nc.vector.tensor_tensor(dnum[:], knn[:], cos[:], ALU.mult)
            den = sm.tile([BS, 1], F32, tag="s1")
            nc.vector.tensor_reduce(den[:], dnum[:], mybir.AxisListType.X, ALU.add)
            rden = sm.tile([BS, 1], F32, tag="s2")
            nc.vector.reciprocal(rden[:], den[:])

            pens = ppc.tile([BS, C], F32, tag="pcos")
            for e_idx in range(E):
                dg = dgp.tile([BS, BS], F32R)
                nc.vector.tensor_scalar(dg[:], iden[:], sn[:, e_idx:e_idx + 1],
                                        None, ALU.mult)
                nc.tensor.matmul(pens[:], dg[:],
                                 lstore[:, e_idx * C:(e_idx + 1) * C],
                                 start=(e_idx == 0), stop=(e_idx == E - 1))
            ens = wk.tile([BS, C], F32, tag="van")
            nc.scalar.activation(ens[:], pens[:], A.Copy, scale=rden[:])
            nc.sync.dma_start(o_ens[:, :], ens[:])

    nc.compile()
    return nc


_NC_CACHE = {}


def _get_nc(with_expert_bias):
    if with_expert_bias not in _NC_CACHE:
        _NC_CACHE[with_expert_bias] = build_nc(with_expert_bias)
    return _NC_CACHE[with_expert_bias]


def kernel(x, keys, expert_w, expert_b, vanilla_w, vanilla_b, tanh_w, tanh_b):
    global LAST_RESULTS
    from concourse.bass_utils import run_bass_kernel_spmd

    f32 = np.float32
    x = np.ascontiguousarray(x, dtype=f32)
    keysT = np.ascontiguousarray(np.asarray(keys, dtype=f32).T)
    ewT = np.ascontiguousarray(np.asarray(expert_w, dtype=f32).transpose(0, 2, 1))
    eb = np.ascontiguousarray(expert_b, dtype=f32)
    vwT = np.ascontiguousarray(np.asarray(vanilla_w, dtype=f32).T)
    vb = np.ascontiguousarray(vanilla_b, dtype=f32).reshape(1, C)
    twT = np.ascontiguousarray(np.asarray(tanh_w, dtype=f32).T)
    tb = np.ascontiguousarray(tanh_b, dtype=f32).reshape(1, C)
    iden10 = np.ascontiguousarray(np.eye(BS, dtype=f32) * f32(10.0))

    in_maps = []
    for i in range(NCORES):
        xs = np.ascontiguousarray(x[i * BS:(i + 1) * BS])
        in_maps.append({
            "xT": np.ascontiguousarray(xs.T),
            "x": xs,
            "keysT": keysT,
            "ewT": ewT,
            "eb": eb,
            "vwT": vwT,
            "vb": vb,
            "twT": twT,
            "tb": tb,
            "iden10": iden10,
        })

    nc = _get_nc(bool(np.any(eb)))
    trace = bool(int(os.environ.get("KERNEL_TRACE", "0")))
    res = run_bass_kernel_spmd(nc, in_maps, core_ids=list(range(NCORES)),
                               trace=trace)
    LAST_RESULTS = res

    ens = np.concatenate([res.results[i]["o_ens"] for i in range(NCORES)], axis=0)
    tout = np.concatenate([res.results[i]["o_tanh"] for i in range(NCORES)], axis=0)
    van = np.concatenate([res.results[i]["o_van"] for i in range(NCORES)], axis=0)
    cdist = np.concatenate([res.results[i]["o_cd"] for i in range(NCORES)], axis=0)
    knn = np.concatenate([res.results[i]["o_knn"] for i in range(NCORES)], axis=0)
    return (ens, tout, van, cdist, knn)
